# revision 12
# baseline (speedup 1.0000x reference)
"""CFConv (SchNet continuous-filter conv) TRN2 Bass kernel, 8-core row-parallel.

Reference computation (batch=1):
  w1 = silu(e @ w1_w^T + w1_b)        (512, 511, 128)
  w2 = silu(e @ w2_w^T + w2_b)
  xp = silu(x @ phi_w^T + phi_b)      (512, 128)
  x_nbs[i, j] = xp[j + (j >= i)]      neighbor gather
  v  = (concat(xp_i * w1, x_nbs * w2) @ o_w^T + o_b) * mask
  return split(v, 3, axis=-1)

Strategy (SPMD, 64 atom-rows per core, no collectives), redesigned vs the
223.8us fp32 baseline:
  - host pre-scatters e into the atom-indexed (512-wide) layout with the
    diagonal zeroed (neighbor gather becomes elementwise vs xp^T), computes
    xp itself (tiny), and pre-transposes operands; all matmul operands are
    bf16 (measured end-to-end rel err ~8e-3 vs the 2e-2 budget).
  - output layout is TRANSPOSED vs the baseline: out[row, g, atom] with the
    384 output features g on partitions (3 chunks of 128) and atoms on the
    free axis.  This removes the per-row bias matmuls and the on-device
    mask/bias epilogue entirely: the host applies (out + o_b) * mask during
    the unshard gather (it already touches every element there).
  - outputs are stored bf16 (dominant HBM traffic, 25MB/core, halved vs
    fp32); host casts back to fp32.
  - rows are processed in pairs: the 4 filter matmuls of a pair go to two
    distinct PE row-groups back to back (concurrent on HW), write one
    2-bank PSUM tile per filter, and are activated by ONE 1024-wide silu
    per filter (halves ACT per-op overhead).  v2 = w2sil * xp^T is one
    1024-wide bf16 tensor_tensor on DVE (2x packed mode); v1 = w1sil *
    xp_own[i] is a per-partition-scalar multiply on GPSIMD (keeps DVE free
    for the PSUM drains, which only ACT/DVE can do).
  - per row, 3 output chunks: PSUM[128g, 512a] = o_wT1_c^T @ v1T +
    o_wT2_c^T @ v2T; drained psum->sbuf as bf16 split between ACT and DVE,
    then ONE 384KB DMA per row (HWDGE fixed cost dominates small DMAs).
"""

import sys

sys.path.insert(0, "/opt/trn_rl_repo")

import numpy as np  # noqa: E402
import ml_dtypes  # noqa: E402

N_A, N_F, N_K, N_G = 512, 128, 20, 384
NK1 = N_K + 1                # filter contraction incl the bias ones-row
CORES = 8
ROWS = N_A // CORES          # 64 rows per core
R = 4                        # rows per eT group (partition bases 0/32/64/96)
NGRP = ROWS // R
CH = 3                       # output chunks of 128 features

V1_ENGINE = "pool"           # "dve" | "pool" (fold of xpo into o_wT1)
V2_ENGINE = "dve"            # "dve" | "pool"
# PSUM->SBUF drain split: per chunk, how many columns ACT copies (rest DVE)
EPI_ACT_COLS = (512, 0, 0)

BF16NP = ml_dtypes.bfloat16

_STATE = {}


def _build_nc():
    import concourse.bacc as bacc
    import concourse.mybir as mybir
    import concourse.tile as tile

    F32 = mybir.dt.float32
    BF = mybir.dt.bfloat16
    Silu = mybir.ActivationFunctionType.Silu
    Copy = mybir.ActivationFunctionType.Copy
    MUL = mybir.AluOpType.mult

    nc = bacc.Bacc(None)

    d_e = nc.dram_tensor("eT", [NGRP, 128, N_A], BF, kind="ExternalInput")
    d_xpT2 = nc.dram_tensor("xpT2", [N_F, 2 * N_A], BF, kind="ExternalInput")
    d_xpo = nc.dram_tensor("xpo", [N_F, ROWS], F32, kind="ExternalInput")
    d_w1T = nc.dram_tensor("w1T_rep", [128, 128], BF, kind="ExternalInput")
    d_w2T = nc.dram_tensor("w2T_rep", [128, 128], BF, kind="ExternalInput")
    d_oT1 = nc.dram_tensor("o_wT1", [N_F, N_G], BF, kind="ExternalInput")
    d_oT2 = nc.dram_tensor("o_wT2", [N_F, N_G], BF, kind="ExternalInput")
    d_out = nc.dram_tensor("out", [ROWS, N_G, N_A], BF, kind="ExternalOutput")

    with tile.TileContext(nc) as tc:
        with tc.tile_pool(name="static", bufs=1) as st:
            xpT2 = st.tile([N_F, 2 * N_A], BF)
            nc.sync.dma_start(xpT2[:], d_xpT2[:])
            xpo = st.tile([N_F, ROWS], F32)
            nc.sync.dma_start(xpo[:], d_xpo[:])
            w1rep = st.tile([128, 128], BF)
            nc.sync.dma_start(w1rep[:], d_w1T[:])
            w2rep = st.tile([128, 128], BF)
            nc.sync.dma_start(w2rep[:], d_w2T[:])
            o_wT1 = st.tile([N_F, N_G], BF)
            nc.sync.dma_start(o_wT1[:], d_oT1[:])
            o_wT2 = st.tile([N_F, N_G], BF)
            nc.sync.dma_start(o_wT2[:], d_oT2[:])

            with tc.tile_pool(name="loop", bufs=1) as lp, \
                 tc.tile_pool(name="wps", bufs=1, space="PSUM") as wps, \
                 tc.tile_pool(name="ops", bufs=4, space="PSUM") as ops:

                def load_group(g):
                    eT4 = lp.tile([128, N_A], BF, tag="eT4", bufs=4)
                    nc.sync.dma_start(eT4[:], d_e[g])
                    return eT4

                eT4s = {0: load_group(0), 1: load_group(1)}

                def veng(which):
                    return nc.vector if which == "dve" else nc.gpsimd

                NPAIR = NGRP * 2

                def head(p):
                    """Filter MMs + silu + v2 + v1-folds for pair p."""
                    g, half = divmod(p, 2)
                    if g not in eT4s:
                        eT4s[g] = load_group(g)
                    eT4 = eT4s[g]
                    # one 4-bank PSUM tile per pair of rows, laid out
                    # [w1(r0) | w1(r1) | w2(r0) | w2(r1)]; the silu bias
                    # rides in the weights (ones-row in eT, K=21), so a
                    # single bias-free 2048-wide silu activates all four
                    wp = wps.tile([128, 4 * N_A], F32, tag="wp")
                    for r2 in range(2):
                        r = 2 * half + r2
                        nc.tensor.matmul(
                            wp[:, N_A * r2:N_A * (r2 + 1)],
                            w1rep[32 * r:32 * r + NK1, :],
                            eT4[32 * r:32 * r + NK1, :],
                            start=True, stop=True,
                            tile_position=(32 * r, 0))
                    for r2 in range(2):
                        r = 2 * half + r2
                        nc.tensor.matmul(
                            wp[:, N_A * (2 + r2):N_A * (3 + r2)],
                            w2rep[32 * r:32 * r + NK1, :],
                            eT4[32 * r:32 * r + NK1, :],
                            start=True, stop=True,
                            tile_position=(32 * r, 0))
                    ws = lp.tile([128, 4 * N_A], BF, tag="ws", bufs=2)
                    nc.scalar.activation(ws[:], wp[:], Silu)
                    v2p = lp.tile([128, 2 * N_A], BF, tag="v2p", bufs=2)
                    veng(V2_ENGINE).tensor_tensor(
                        v2p[:], ws[:, 2 * N_A:4 * N_A], xpT2[:], MUL)
                    o1fs = []
                    for r2 in range(2):
                        i = R * g + 2 * half + r2
                        # fold xp_own[i] into the v1-path stationary
                        o1f = lp.tile([N_F, N_G], BF, tag="o1f", bufs=4)
                        veng(V1_ENGINE).tensor_scalar_mul(
                            o1f[:], o_wT1[:], xpo[:, i:i + 1])
                        o1fs.append(o1f)
                    if p % 2 == 1 and g + 2 < NGRP:
                        eT4s[g + 2] = load_group(g + 2)
                        eT4s.pop(g, None)
                    return ws, v2p, o1fs

                def body(p, ws, v2p, o1fs):
                    """Output matmuls + psum drain + store for pair p."""
                    g, half = divmod(p, 2)
                    for r2 in range(2):
                        i = R * g + 2 * half + r2
                        sl = slice(N_A * r2, N_A * (r2 + 1))
                        o1f = o1fs[r2]
                        osb = lp.tile([128, CH * N_A], BF, tag="osb", bufs=3)
                        for c in range(CH):
                            op = ops.tile([128, N_A], F32, tag="op")
                            nc.tensor.matmul(
                                op[:], o1f[:, 128 * c:128 * (c + 1)],
                                ws[:, sl], start=True, stop=False)
                            nc.tensor.matmul(
                                op[:], o_wT2[:, 128 * c:128 * (c + 1)],
                                v2p[:, sl], start=False, stop=True)
                            ac = EPI_ACT_COLS[c]
                            if ac > 0:
                                nc.scalar.activation(
                                    osb[:, c * N_A:c * N_A + ac],
                                    op[:, 0:ac], Copy)
                            if ac < N_A:
                                nc.vector.tensor_copy(
                                    osb[:, c * N_A + ac:(c + 1) * N_A],
                                    op[:, ac:N_A])
                        nc.sync.dma_start(
                            d_out[i].rearrange("(c p) a -> p c a", p=128),
                            osb[:].rearrange("p (c a) -> p c a", a=N_A))

                prev = head(0)
                for p in range(1, NPAIR):
                    cur = head(p)
                    body(p - 1, *prev)
                    prev = cur
                body(NPAIR - 1, *prev)

    nc.compile()
    return nc


def _get_state():
    if "nc" not in _STATE:
        _STATE["nc"] = _build_nc()
        # pos->atom index map per core: a = j + (j >= i_abs)
        j = np.arange(N_A - 1)[None, :]
        scat = []
        for c in range(CORES):
            i_abs = (c * ROWS + np.arange(ROWS))[:, None]
            scat.append((j + (j >= i_abs)).astype(np.int64))  # (ROWS, 511)
        _STATE["aidx"] = scat
        _STATE["rows"] = np.arange(ROWS)[:, None]
    return _STATE


def _silu(x):
    return x / (1.0 + np.exp(-x))


def build_in_maps(x, e, w1_w, w1_b, w2_w, w2_b, phi_w, phi_b, o_w):
    """x (512,128), e (512,511,20) fp32 -> per-core in_maps."""
    st = _get_state()
    rows = st["rows"]

    def _rep4(wTb):  # (21,128) -> (128,128) at partition bases 0/32/64/96
        out = np.zeros((128, wTb.shape[1]), np.float32)
        for r in range(4):
            out[32 * r:32 * r + wTb.shape[0]] = wTb
        return out

    x = np.asarray(x, np.float32)
    o_wn = np.asarray(o_w, np.float32)             # (384, 256)
    xp = _silu(x @ np.asarray(phi_w, np.float32).T
               + np.asarray(phi_b, np.float32))    # (512, 128)
    xpT = np.ascontiguousarray(xp.T).astype(BF16NP)
    # filter weights with the silu bias as contraction row 20 (eT ones-row)
    w1Tb = np.vstack([np.asarray(w1_w, np.float32).T,
                      np.asarray(w1_b, np.float32).reshape(1, N_F)])
    w2Tb = np.vstack([np.asarray(w2_w, np.float32).T,
                      np.asarray(w2_b, np.float32).reshape(1, N_F)])
    shared = {
        "xpT2": np.concatenate([xpT, xpT], axis=1),              # (128,1024)
        "w1T_rep": _rep4(w1Tb).astype(BF16NP),
        "w2T_rep": _rep4(w2Tb).astype(BF16NP),
        "o_wT1": np.ascontiguousarray(o_wn[:, 0:N_F].T).astype(BF16NP),
        "o_wT2": np.ascontiguousarray(o_wn[:, N_F:2 * N_F].T).astype(BF16NP),
    }

    in_maps = []
    for c in range(CORES):
        sl = slice(c * ROWS, (c + 1) * ROWS)
        aidx = st["aidx"][c]
        e_at = np.zeros((ROWS, N_A, N_K), np.float32)
        e_at[rows, aidx] = e[sl]
        # eT[g, 32r+k, n] = e_at[4g+r, n, k]; row 20 = 1.0 (bias)
        eT = np.zeros((NGRP, 128, N_A), BF16NP)
        eT.reshape(NGRP, R, 32, N_A)[:, :, 0:N_K] = \
            e_at.reshape(NGRP, R, N_A, N_K).transpose(0, 1, 3, 2)
        eT.reshape(NGRP, R, 32, N_A)[:, :, N_K] = 1.0
        in_maps.append({
            "eT": eT,
            "xpo": np.ascontiguousarray(xp[sl].T),               # (128, 64)
            **shared,
        })
    return in_maps


def kernel(x, e, mask, loop_mask, w1_w, w1_b, w2_w, w2_b, phi_w, phi_b, o_w, o_b):
    st = _get_state()
    from concourse.bass_utils import run_bass_kernel_spmd

    x = np.asarray(x, np.float32)[0]                                # (512,128)
    e = np.asarray(e, np.float32)[0]                                # (512,511,20)
    mask_f = np.asarray(mask, np.float32)[0, :, :, 0]               # (512,511)
    in_maps = build_in_maps(x, e, w1_w, w1_b, w2_w, w2_b,
                            phi_w, phi_b, o_w)

    res = run_bass_kernel_spmd(st["nc"], in_maps, list(range(CORES)))

    rows = st["rows"]
    ob = np.asarray(o_b, np.float32).reshape(1, 1, N_G)
    parts = []
    for c in range(CORES):
        out_at = res.results[c]["out"]                  # (ROWS, 384, 512) bf16
        # -> (ROWS, 511, 384) fp32, gathered back to neighbor order
        vt = out_at.astype(np.float32).transpose(0, 2, 1)
        parts.append(vt[rows, st["aidx"][c]])
    v = np.concatenate(parts, axis=0)[None]             # (1, 512, 511, 384)
    v += ob
    v *= mask_f[None, :, :, None]
    s1, s2, s3 = np.split(v, 3, axis=-1)
    return (s1, s2, s3)
